# revision 17
# baseline (speedup 1.0000x reference)
"""Copy-enhanced CodeT5 head (histogram/scatter blend) on 8 TRN2 NeuronCores.

Strategy: data-parallel over (batch, T/2) -> 8 shards of 128 decoder rows.
All HBM traffic in bf16 (host casts at the shard boundary, upcasts on the
way out).  Per core, for its [128, V] output block:

  phase 1 (DMA-bound, ~34us):
    loads alternate the two HWDGE queues, big tiles first
    A^T      = sum_h xattn^T[h]   (DVE tree in place on host-pretransposed tile)
    p_gen    = 1/(1+exp(-( (A@enc)@w1/H + dec.w2 + b )))  (PE + DVE + ACT exp)
    exp,Z    = exp(logits) in place over the streamed bf16 tiles, row sums
               via ACT accumulators
    P_copy   = two scatter-adds (lo/hi vocab half) of masked duplicate-combined
               attention mass into a zeroed bf16 pair-packed accumulator.
               Scatter indices are plain pair ids (host-wrapped layout);
               duplicate occurrences carry zero value via a first-occurrence
               mask, so no dump-slot redirect chain is needed.  The zeroing
               is split across ACT/DVE/GPSIMD slices, all off-critical-path.
  phase 2 (~26us):
    out      = exp * (p_gen/Z) + P_copy: per-partition scale in place
               (ACT scalar.mul / DVE tensor_scalar split), DVE tensor_add
               into otile, stores on sync/scalar queues.  Low-half chunks
               blend as soon as the lo scatter lands.

No collectives: every core owns a disjoint output block.
"""
import sys

sys.path.insert(0, "/opt/trn_rl_repo")

import ml_dtypes
import numpy as np

import concourse.bass as bass  # noqa: F401  (registers engine classes)
import concourse.mybir as mybir
from concourse import bacc, bass_utils
from concourse.tile import TileContext

B, S, T, D, H, V = 4, 512, 256, 1024, 16, 32105
P = 128
NCORES = 8
VPAD = 32768                # padded vocab width (16 x 2048)
NPAIR = VPAD // 2           # 16384 pair slots
HPAIR = NPAIR // 2          # 8192: lo/hi scatter halves
# load tiles: 6 x 4096 + 4 x 2048 (small tail tiles shrink the exp tail)
LSIZES = [4096] * 6 + [2048] * 4
LOFFS = [sum(LSIZES[:i]) for i in range(len(LSIZES))]
NLT = len(LSIZES)
OTILE = 2048
NOT = VPAD // OTILE         # 16 blend/store chunks
# chunks whose scale pass runs on ACT (rest on DVE); adds on gpsimd for GP_TT
ACT_SET = frozenset([1, 5, 9, 13])
GP_TT = frozenset([3, 7, 11, 15])
# pcopy zeroing split (uint32 columns of the [P, 16384] u32 view)
ZSPLIT = (5456, 10912)      # [0:a]=ACT, [a:b]=DVE, [b:16384]=GPSIMD

AluOp = mybir.AluOpType
Act = mybir.ActivationFunctionType
f32 = mybir.dt.float32
bf16 = mybir.dt.bfloat16
i32 = mybir.dt.int32
i16 = mybir.dt.int16
u32 = mybir.dt.uint32
BF = ml_dtypes.bfloat16


def _body(tc, ids_bc_d, ids_col_d, idx_lo_d, logits_d, xattn_d,
          enc_d, dec_d, w1_d, w2_d, wb_d, out_d):
    nc = tc.nc
    with tc.tile_pool(name="fix", bufs=1) as fix, \
         tc.tile_pool(name="work", bufs=2) as work, \
         tc.tile_pool(name="opool", bufs=2) as opool, \
         tc.tile_pool(name="psum", bufs=1, space="PSUM") as psum:

        # ---- persistent tiles ----
        # separate logits tiles (one big tile would WAW-serialize the loads)
        ltiles = [fix.tile([P, LSIZES[k]], bf16, name=f"lt{k}")
                  for k in range(NLT)]
        pcopy = fix.tile([P, NPAIR, 2], bf16)
        pcopy_u32 = pcopy[:].rearrange("p a b -> p (a b)").bitcast(u32)

        # ---- input DMAs: the scalar engine issues only two logits tiles so
        # its sequencer reaches the exp stream almost immediately (each issue
        # costs ~0.6us and can block on completion-semaphore reuse) ----
        nc.scalar.dma_start(out=ltiles[1][:], in_=logits_d[1][:, :LSIZES[1]])
        nc.scalar.dma_start(out=ltiles[3][:], in_=logits_d[3][:, :LSIZES[3]])
        ids_bc_i = fix.tile([P, S], i32)
        ids_col_i = fix.tile([P, 4], i32)
        idx_lo = fix.tile([P, 32], i16)
        wb_bc = fix.tile([P, 1], f32)
        xattnT = fix.tile([P, 4, H, P], bf16)
        w1b = fix.tile([P, D], bf16)
        w2b = fix.tile([P, D], bf16)
        enc_t = fix.tile([P, 4, D], bf16)
        dec_t = fix.tile([P, D], bf16)
        nc.sync.dma_start(out=ltiles[0][:], in_=logits_d[0][:, :LSIZES[0]])
        nc.sync.dma_start(out=ids_bc_i[:], in_=ids_bc_d[:])
        nc.sync.dma_start(out=ids_col_i[:], in_=ids_col_d[:])
        nc.sync.dma_start(out=idx_lo[:], in_=idx_lo_d[:])
        nc.sync.dma_start(out=wb_bc[:], in_=wb_d[:])
        nc.sync.dma_start(out=xattnT[:], in_=xattn_d[:])
        nc.sync.dma_start(out=ltiles[2][:], in_=logits_d[2][:, :LSIZES[2]])
        nc.sync.dma_start(out=enc_t[:], in_=enc_d[:])
        nc.sync.dma_start(out=dec_t[:], in_=dec_d[:])
        nc.sync.dma_start(out=w1b[:], in_=w1_d[:])
        nc.sync.dma_start(out=w2b[:], in_=w2_d[:])
        for k in range(4, NLT):
            nc.sync.dma_start(out=ltiles[k][:], in_=logits_d[k][:, :LSIZES[k]])

        # ---- zero the scatter accumulator: DVE/GPSIMD halves, both early ----
        nc.vector.memset(pcopy_u32[:, 0:NPAIR // 2], 0)
        nc.gpsimd.memset(pcopy_u32[:, NPAIR // 2:NPAIR], 0)

        # ---- exp stream on ACT (in place over the loaded bf16 logits) ----
        zparts = fix.tile([P, NLT], f32)

        def _exps(ks):
            for k in ks:
                sl = ltiles[k][:]
                nc.scalar.activation(out=sl, in_=sl, func=Act.Exp,
                                     accum_out=zparts[:, k:k + 1])

        _exps(range(0, 3))

        # ---- pair/Sel machinery (needs only ids) ----
        pair_bi = work.tile([P, S], i32, tag="wk")
        nc.vector.tensor_scalar(pair_bi[:], ids_bc_i[:], 1, None,
                                AluOp.arith_shift_right)
        pair_bc = fix.tile([P, S], f32)
        nc.vector.tensor_copy(out=pair_bc[:], in_=pair_bi[:])
        parity_ci = fix.tile([P, 4], i32)
        nc.vector.tensor_scalar(parity_ci[:], ids_col_i[:], 1, None,
                                AluOp.bitwise_and)
        parity_col = fix.tile([P, 4], f32)
        nc.vector.tensor_copy(out=parity_col[:], in_=parity_ci[:])
        pair_ci = fix.tile([P, 4], i32)
        nc.vector.tensor_scalar(pair_ci[:], ids_col_i[:], 1, None,
                                AluOp.arith_shift_right)
        pair_col = fix.tile([P, 4], f32)
        nc.vector.tensor_copy(out=pair_col[:], in_=pair_ci[:])
        par_is = fix.tile([P, 4, 2], f32)
        nc.vector.tensor_scalar(par_is[:, :, 0], parity_col[:], 0.0, None,
                                AluOp.is_equal)
        nc.vector.tensor_scalar(par_is[:, :, 1], parity_col[:], 1.0, None,
                                AluOp.is_equal)
        Sel = fix.tile([P, 4, S], bf16)
        for kk in range(4):
            nc.vector.tensor_scalar(Sel[:, kk, :], pair_bc[:],
                                    pair_col[:, kk:kk + 1], None, AluOp.is_equal)
        ones_c = fix.tile([P, P], bf16)
        nc.vector.memset(ones_c[:], 1.0)

        # lower-triangular mask (strictly s' < s) into a SEPARATE tile so the
        # dup-detect chain doesn't wait for the m2 reads of Sel
        LSel = fix.tile([P, 4, S], bf16)
        for kk in range(4):
            nc.gpsimd.affine_select(
                out=LSel[:, kk, :], in_=Sel[:, kk, :],
                pattern=[[1, S]], compare_op=AluOp.is_ge, fill=0.0,
                base=-(kk * P) - 1, channel_multiplier=-1,
            )

        # ---- p_lin2 = dec . w2 (dec-only; emitted before the head tree) ----
        pl2 = fix.tile([P, 1], f32)
        junk2 = work.tile([P, D], bf16, tag="jnk", bufs=2)
        nc.vector.scalar_tensor_tensor(out=junk2[:], in0=dec_t[:], scalar=1.0,
                                       in1=w2b[:], op0=AluOp.mult, op1=AluOp.mult,
                                       accum_out=pl2[:])

        # ---- head sum -> A^T directly (xattnT is [s, kk, h, t]); the tree
        # collapses in place over xattnT (dead afterwards) to save SBUF ----
        nc.vector.tensor_add(out=xattnT[:, :, 0:8, :], in0=xattnT[:, :, 0:8, :],
                             in1=xattnT[:, :, 8:16, :])
        nc.vector.tensor_add(out=xattnT[:, :, 0:4, :], in0=xattnT[:, :, 0:4, :],
                             in1=xattnT[:, :, 4:8, :])
        nc.vector.tensor_add(out=xattnT[:, :, 0:2, :], in0=xattnT[:, :, 0:2, :],
                             in1=xattnT[:, :, 2:4, :])
        A_T = fix.tile([P, 4, P], bf16)
        nc.vector.tensor_add(out=A_T[:], in0=xattnT[:, :, 0, :],
                             in1=xattnT[:, :, 1, :])

        # ---- PE: h* = A@enc (split at the PSUM bank boundary), dup counts ----
        hstar_a = psum.tile([P, D // 2], f32, tag="hstara")
        hstar_b = psum.tile([P, D // 2], f32, tag="hstarb")
        for kk in range(4):
            nc.tensor.matmul(hstar_a[:], A_T[:, kk, :], enc_t[:, kk, 0:D // 2],
                             start=(kk == 0), stop=(kk == 3))
        for kk in range(4):
            nc.tensor.matmul(hstar_b[:], A_T[:, kk, :], enc_t[:, kk, D // 2:D],
                             start=(kk == 0), stop=(kk == 3))
        # ---- p_gen: (A@enc)@w1/H + dec.w2 + b, sigmoid via exp+recip ----
        pl1a = fix.tile([P, 1], f32)
        junk1 = work.tile([P, D // 2], bf16, tag="jnk", bufs=2)
        nc.vector.scalar_tensor_tensor(out=junk1[:], in0=hstar_a[:], scalar=1.0,
                                       in1=w1b[:, 0:D // 2], op0=AluOp.mult,
                                       op1=AluOp.mult, accum_out=pl1a[:])
        pl1 = fix.tile([P, 1], f32)
        junk1b = work.tile([P, D // 2], bf16, tag="jnk", bufs=2)
        nc.vector.scalar_tensor_tensor(out=junk1b[:], in0=hstar_b[:], scalar=1.0,
                                       in1=w1b[:, D // 2:D], op0=AluOp.mult,
                                       op1=AluOp.mult, accum_out=pl1[:])
        nc.vector.tensor_add(out=pl1[:], in0=pl1[:], in1=pl1a[:])
        neg2b = fix.tile([P, 1], f32)
        nc.vector.scalar_tensor_tensor(out=neg2b[:], in0=pl2[:], scalar=-1.0,
                                       in1=wb_bc[:], op0=AluOp.mult,
                                       op1=AluOp.subtract)
        e_t = fix.tile([P, 1], f32)
        nc.scalar.activation(out=e_t[:], in_=pl1[:], func=Act.Exp,
                             bias=neg2b[:], scale=-1.0 / H)
        # dup count broadcast to every partition via an all-ones stationary
        dup_ps = psum.tile([P, S], f32, tag="dup")
        for kk in range(4):
            nc.tensor.matmul(dup_ps[:], ones_c[:], LSel[:, kk, :],
                             start=(kk == 0), stop=(kk == 3))

        # ---- duplicate-combine matmuls (both parity lanes) ----
        comb_e = psum.tile([P, S], f32, tag="combe")
        comb_o = psum.tile([P, S], f32, tag="combo")
        for lane, comb_ps_l in ((0, comb_e), (1, comb_o)):
            for kk in range(4):
                m2 = work.tile([P, S], bf16, tag="m2", bufs=2)
                nc.vector.tensor_scalar(m2[:], Sel[:, kk, :],
                                        par_is[:, kk:kk + 1, lane], None,
                                        AluOp.mult)
                nc.tensor.matmul(comb_ps_l[:], A_T[:, kk, :], m2[:],
                                 start=(kk == 0), stop=(kk == 3))

        _exps(range(3, NLT))
        pe1 = fix.tile([P, 1], f32)
        nc.vector.tensor_scalar(pe1[:], e_t[:], 1.0, None, AluOp.add)
        p_gen = fix.tile([P, 1], f32)
        nc.vector.reciprocal(out=p_gen[:], in_=pe1[:])
        s1 = fix.tile([P, 1], f32)
        nc.vector.tensor_scalar(s1[:], p_gen[:], -1.0 / H, 1.0 / H,
                                AluOp.mult, AluOp.add)

        # ---- scatter adds: values masked by the first-occurrence plane ----
        first_pc = fix.tile([P, S], f32)
        nc.vector.tensor_scalar(first_pc[:], dup_ps[:], 0.0, None, AluOp.is_equal)
        # compact the psum combine results (pre-scaled by s1) into SBUF
        ce = fix.tile([P, S], f32)
        nc.vector.tensor_scalar(ce[:], comb_e[:], s1[:], None, AluOp.mult)
        co = fix.tile([P, S], f32)
        nc.vector.tensor_scalar(co[:], comb_o[:], s1[:], None, AluOp.mult)
        add_p = fix.tile([P, S, 2], bf16)
        nc.vector.scalar_tensor_tensor(
            out=add_p[:, :, 0], in0=ce[:], scalar=1.0, in1=first_pc[:],
            op0=AluOp.mult, op1=AluOp.mult)
        nc.vector.scalar_tensor_tensor(
            out=add_p[:, :, 1], in0=co[:], scalar=1.0, in1=first_pc[:],
            op0=AluOp.mult, op1=AluOp.mult)
        nc.gpsimd.scatter_add(in_ap=pcopy[:], idxs_ap=idx_lo[:],
                              add_ap=add_p[:], channels=P, num_elems=NPAIR,
                              d=2, num_idxs=S)

        # ---- softmax scale ----
        Z = fix.tile([P, 1], f32)
        nc.vector.tensor_reduce(out=Z[:], in_=zparts[:], axis=mybir.AxisListType.X,
                                op=AluOp.add)
        invZ = fix.tile([P, 1], f32)
        nc.vector.reciprocal(out=invZ[:], in_=Z[:])
        s0 = fix.tile([P, 1], f32)
        nc.vector.tensor_mul(out=s0[:], in0=p_gen[:], in1=invZ[:])

        # ---- phase 2: scale (ACT or DVE, in place) + add + store ----
        pcopy_flat = pcopy[:].rearrange("p a b -> p (a b)")
        for k in range(NOT):
            off = k * OTILE
            if k < 12:
                sl = ltiles[k // 2][:, (k % 2) * OTILE:(k % 2) * OTILE + OTILE]
            else:
                sl = ltiles[6 + (k - 12)][:]
            if k in ACT_SET:
                nc.scalar.mul(sl, sl, s0[:])
            else:
                nc.vector.tensor_scalar(sl, sl, s0[:], None, AluOp.mult)
            otile = opool.tile([P, OTILE], bf16, tag="ot", name=f"ot{k}")
            add_eng = nc.gpsimd if k in GP_TT else nc.vector
            add_eng.tensor_add(out=otile[:], in0=sl,
                               in1=pcopy_flat[:, off:off + OTILE])
            eng = nc.sync if k % 2 == 0 else nc.scalar
            eng.dma_start(out=out_d[k], in_=otile[:])


_CACHE = {}


def _get_graph():
    if "nc" in _CACHE:
        return _CACHE["nc"]
    nc = bacc.Bacc("TRN2", target_bir_lowering=False, debug=False,
                   num_devices=NCORES)
    ids_bc_d = nc.dram_tensor("ids_bc", [P, S], i32, kind="ExternalInput").ap()
    ids_col_d = nc.dram_tensor("ids_col", [P, 4], i32, kind="ExternalInput").ap()
    idx_lo_d = nc.dram_tensor("idx_lo", [P, 32], i16, kind="ExternalInput").ap()
    logits_d = nc.dram_tensor("logits", [NLT, P, 4096], bf16,
                              kind="ExternalInput").ap()
    xattn_d = nc.dram_tensor("xattn", [P, 4, H, P], bf16,
                             kind="ExternalInput").ap()
    enc_d = nc.dram_tensor("enc", [P, 4, D], bf16, kind="ExternalInput").ap()
    dec_d = nc.dram_tensor("dec", [P, D], bf16, kind="ExternalInput").ap()
    w1_d = nc.dram_tensor("w1", [P, D], bf16, kind="ExternalInput").ap()
    w2_d = nc.dram_tensor("w2", [P, D], bf16, kind="ExternalInput").ap()
    wb_d = nc.dram_tensor("wb", [P, 1], f32, kind="ExternalInput").ap()
    out_d = nc.dram_tensor("out", [NOT, P, OTILE], bf16,
                           kind="ExternalOutput").ap()
    with TileContext(nc) as tc:
        _body(tc, ids_bc_d, ids_col_d, idx_lo_d, logits_d, xattn_d,
              enc_d, dec_d, w1_d, w2_d, wb_d, out_d)
    nc.compile()
    _CACHE["nc"] = nc
    return nc


def _retile_logits(block):
    # [P, V] f32 -> [NLT, P, 4096] bf16; tile k occupies [:, :LSIZES[k]]
    out = np.full((NLT, P, 4096), -100.0, BF)
    for k in range(NLT):
        off = LOFFS[k]
        w = min(LSIZES[k], max(0, V - off))
        if w > 0:
            out[k, :, :w] = block[:, off:off + w].astype(BF)
    return out


def _wrap16(vals):
    # [S] -> [128, 32] i16 in the scatter_add wrapped layout: unwrapped
    # position J reads idx element [p=J%16, i=J//16]; replicated x8 groups
    w = np.zeros((16, 32), np.int16)
    v = np.asarray(vals, np.int16)
    for j in range(S):
        w[j % 16, j // 16] = v[j]
    return np.ascontiguousarray(np.tile(w, (8, 1)))


def _route_full(pair):
    """Scatter index row: first occurrence targets its real pair slot; every
    duplicate is parked on a unique unused slot.  The device-side
    first-occurrence mask zeroes the parked entries' values, and unique slots
    mean the non-accumulating scatter pipeline never races a slot."""
    idx = np.full(S, -1, np.int64)
    seen = set()
    for j, p in enumerate(pair):
        if p not in seen:
            seen.add(int(p))
            idx[j] = p
    free = (s for s in range(NPAIR) if s not in seen)
    for j in range(S):
        if idx[j] < 0:
            idx[j] = next(free)
    return idx


def _shard(inputs):
    ids = np.asarray(inputs["input_ids"])
    logits = np.asarray(inputs["logits"], dtype=np.float32)
    enc = np.asarray(inputs["encoder_hidden_states"], dtype=np.float32)
    dec = np.asarray(inputs["decoder_hidden_states"], dtype=np.float32)
    xattn = np.asarray(inputs["cross_attentions"], dtype=np.float32)
    wgw = np.asarray(inputs["W_gen_w"], dtype=np.float32)
    wgb = np.asarray(inputs["W_gen_b"], dtype=np.float32)
    w1 = np.ascontiguousarray(np.broadcast_to(wgw[0:1, 0:D], (P, D))).astype(BF)
    w2 = np.ascontiguousarray(
        np.broadcast_to(wgw[0:1, D:2 * D], (P, D))).astype(BF)
    wb = np.full((P, 1), wgb[0], np.float32)
    in_maps = []
    for c in range(NCORES):
        b, th = c // 2, c % 2
        t0 = th * P
        ids_b = ids[b].astype(np.int64)
        pair = (ids_b >> 1).astype(np.int64)
        idx_row = _route_full(pair)
        # xattn [H, 128(t), S] -> [s, h, t] -> [s-in-chunk(128), kk(4), h, t]
        xa = np.transpose(xattn[b, :, t0:t0 + P, :], (2, 0, 1))  # [S, H, P]
        xa = np.ascontiguousarray(
            xa.reshape(4, P, H, P).transpose(1, 0, 2, 3)).astype(BF)
        en = np.ascontiguousarray(
            enc[b].reshape(4, P, D).transpose(1, 0, 2)).astype(BF)
        in_maps.append({
            "ids_bc": np.ascontiguousarray(
                np.broadcast_to(ids_b[None, :], (P, S))).astype(np.int32),
            "ids_col": np.ascontiguousarray(
                ids_b.reshape(4, P).T).astype(np.int32),
            "idx_lo": _wrap16(idx_row),
            "logits": _retile_logits(logits[b, t0:t0 + P, :]),
            "xattn": xa,
            "enc": en,
            "dec": np.ascontiguousarray(dec[b, t0:t0 + P, :]).astype(BF),
            "w1": w1,
            "w2": w2,
            "wb": wb,
        })
    return in_maps


def run(inputs, trace=False):
    nc = _get_graph()
    in_maps = _shard(inputs)
    res = bass_utils.run_bass_kernel_spmd(nc, in_maps,
                                          core_ids=list(range(NCORES)),
                                          trace=trace)
    out = np.empty((B, T, V), np.float32)
    for c in range(NCORES):
        b, th = c // 2, c % 2
        tiles = np.asarray(res.results[c]["out"])  # [NOT, P, OTILE] bf16
        block = np.transpose(tiles, (1, 0, 2)).reshape(P, NOT * OTILE)[:, :V]
        out[b, th * P:(th + 1) * P, :] = block.astype(np.float32)
    return out, res


def kernel(**inputs):
    out, _ = run(inputs, trace=False)
    return out
